# revision 31
# baseline (speedup 1.0000x reference)
"""CrossViewFusion Trainium2 kernel (data-parallel over B=8 across 8 cores).

Per batch element:
  x1s = sum_pool4x4(x1)            [C,1024]  (pool /16 folded into Wk,Wv)
  qT  = x2f^T @ (Wq/32)^T          [1024,C]  (1/h attn scale folded into Wq)
  kT  = x1s^T @ (Wk/16)^T          [1024,C]
  v   = (Wv/16) @ x1s              [C,1024]
  aT  = exp(kT^T-contract-qT)      [C1,C2]   (softmax denom via ones-matmul,
  out = (aT^T @ v) * (1/s) + x2               applied post-GEMM as scale)

x1 is cast to bf16 on the host (halves the dominant HBM stream); weights are
pre-scaled, pre-transposed, and shipped in fp8e4 pair-interleaved layout.
All five GEMMs run as fp8 DoubleRow matmuls (two 128-deep contraction slices
per instruction), with power-of-2 rescales folded into the ACT psum->sbuf
converts so no fp8 operand lands in subnormal range.  The attention term is
~1% of the output magnitude, so fp8 staging keeps rel-err well under the
2e-2 gate.

Stream order is row-group-major (all 6 channel tiles per row group), so k is
fully contracted per n-tile as the stream lands.  Pooling runs as a
bf16 add-tree: row-pair stages hit the DVE 2x mode (GpSimd takes the two
earliest channel tiles' stages), and a final 1x pw-reduce writes fp8.  The
attn GEMM accumulates nt0..5 mid-stream; the partial is folded back into
PSUM via an identity-matmul so only nt6/nt7 + exp + out-GEMM sit in the
tail.  The last two row groups are half-size (16 rows) to shrink that tail.
"""

import sys
from contextlib import ExitStack

if "/opt/trn_rl_repo" not in sys.path:
    sys.path.insert(0, "/opt/trn_rl_repo")

import numpy as np
import ml_dtypes

import concourse.bass as bass
import concourse.tile as tile
from concourse import bacc, bass_utils, masks, mybir

FP32 = mybir.dt.float32
BF16 = mybir.dt.bfloat16
FP8 = mybir.dt.float8e4
DRM = mybir.MatmulPerfMode.DoubleRow
AX = mybir.AxisListType
AF = mybir.ActivationFunctionType

NCORES = 8

C = 768            # channels (C1 == C2)
P = 128            # partition size
CT = C // P        # channel tiles (6)
QT = CT // 2       # channel-tile pairs (3)
HW = 32            # pooled spatial side
N = HW * HW        # pooled spatial size (1024)
NT = N // P        # n-tiles (8); one n-tile == 16 source rows
SRC = 128          # source spatial side of x1
POOL = 4
# row groups of the x1 stream: 64 + 32 rows, then 2 x 16 rows (short tail)
GROUPS = [(0, 4), (4, 2), (6, 1), (7, 1)]  # (first nt, n nts)
# fp8 scale plan: shipped wk,wv = W.T/16*64, wq = W.T/32*64. ACT rescales:
SCL_K = 0.25       # kT_stored = 16*k   (sigma ~4 in fp8)
SCL_Q = 0.5        # qT_stored = 32*q   (sigma ~1)
SCL_V = 0.25       # v_stored  = 16*v   (sigma ~4)
SCL_E = 1.0 / 512  # attn psum = 512 * true logits -> exp(psum/512)
ONES_V = 16.0      # colsum ones = 16 -> pss = 16*s -> rcp = 1/(16 s)


def _splits(total, bank=512):
    off, out = 0, []
    while off < total:
        w = min(bank, total - off)
        out.append((off, w))
        off += w
    return out


def build_program(reps=1, loop_reps=None, timing_mode=False):
    """reps: python-unrolled repetitions. loop_reps: on-device For_i repetitions
    (for timing; same program size regardless of trip count). timing_mode makes
    the inputs Internal DRAM (uninitialized, nothing shipped per call)."""
    nc = bacc.Bacc("TRN2", target_bir_lowering=False, debug=False)

    kind = "Internal" if timing_mode else "ExternalInput"
    x1_d = nc.dram_tensor("x1", [C, SRC, SRC], FP8, kind=kind).ap()
    wts_d = nc.dram_tensor("wts", [P, 3 * CT * C], FP8, kind=kind).ap()
    x2_d = nc.dram_tensor("x2b", [P, CT * N], BF16, kind=kind).ap()
    out_d = nc.dram_tensor("out", [C, N], BF16, kind="ExternalOutput").ap()

    with tile.TileContext(nc) as tc:
        with ExitStack() as ctx:
            ent = ctx.enter_context
            const_pool = ent(tc.tile_pool(name="const", bufs=1))
            wts_pool = ent(tc.tile_pool(name="wts", bufs=1))
            x2_pool = ent(tc.tile_pool(name="x2p", bufs=1))
            x2q_pool = ent(tc.tile_pool(name="x2q", bufs=1))
            stream_pool = ent(tc.tile_pool(name="stream", bufs=8))
            tA_pool = ent(tc.tile_pool(name="tA", bufs=2))
            tB_pool = ent(tc.tile_pool(name="tB", bufs=2))
            x1sb_pool = ent(tc.tile_pool(name="x1sb", bufs=1))
            qT_pool = ent(tc.tile_pool(name="qT", bufs=1))
            kT_pool = ent(tc.tile_pool(name="kT", bufs=1))
            v_pool = ent(tc.tile_pool(name="vp", bufs=1))
            attnA_pool = ent(tc.tile_pool(name="attnA", bufs=CT))
            expT_pool = ent(tc.tile_pool(name="expT", bufs=1))
            rcp_pool = ent(tc.tile_pool(name="rcp", bufs=CT))
            out_pool = ent(tc.tile_pool(name="ost", bufs=4))
            ps_wide = ent(tc.tile_pool(name="ps_wide", bufs=2, space="PSUM"))
            ps_half = ent(tc.tile_pool(name="ps_half", bufs=2, space="PSUM"))
            ps_pool = ent(tc.tile_pool(name="ps_pool", bufs=2, space="PSUM"))

            ident = const_pool.tile([P, P], BF16)
            masks.make_identity(nc, ident[:])
            ident8 = const_pool.tile([P, P], FP8)
            nc.scalar.activation(ident8[:], ident[:], AF.Copy)
            id8p = const_pool.tile([P, 2 * P], FP8)
            nc.scalar.activation(id8p[:, 0:P], ident[:], AF.Copy)
            nc.scalar.activation(id8p[:, P:2 * P], ident[:], AF.Copy)
            onesp = const_pool.tile([P, 32], FP8)
            nc.gpsimd.memset(onesp[:], ONES_V)

            def pair(t, width):
                return t[:].rearrange("p (j x) -> p j x", j=2, x=width)

            def body():
                wts = wts_pool.tile([P, 3 * CT * C], FP8)
                nc.scalar.dma_start(wts[:], wts_d)
                x2b6 = x2_pool.tile([P, CT * N], BF16)
                nc.scalar.dma_start(x2b6[:], x2_d)
                x2b = [x2b6[:, ct * N:(ct + 1) * N] for ct in range(CT)]

                def wpair(w, qp):
                    off = (w * QT + qp) * 2 * C
                    return wts[:, off:off + 2 * C].rearrange(
                        "p (j c) -> p j c", j=2)
                wKp = [wpair(0, qp) for qp in range(QT)]
                wVp = [wpair(1, qp) for qp in range(QT)]
                wQp = [wpair(2, qp) for qp in range(QT)]

                # fp8 pair-interleaved copy of x2 for the q GEMM
                x2q6 = x2q_pool.tile([P, CT * N], FP8)
                for ct in range(CT):
                    nc.scalar.activation(
                        x2q6[:, (ct // 2) * 2 * N + (ct % 2) * N:
                             (ct // 2) * 2 * N + (ct % 2 + 1) * N],
                        x2b[ct], AF.Copy)
                x2q = [x2q6[:, qp * 2 * N:(qp + 1) * 2 * N]
                       for qp in range(QT)]

                qTp = [qT_pool.tile([P, 2 * C], FP8, name=f"qT{i}")
                       for i in range(NT // 2)]
                kTp = [kT_pool.tile([P, 2 * C], FP8, name=f"kT{i}")
                       for i in range(NT // 2)]
                x1sq = [x1sb_pool.tile([P, 2 * N], FP8, name=f"x1s{qp}")
                        for qp in range(QT)]
                x1sq6 = [x1sb_pool.tile([P, 2 * P], FP8, name=f"x1s6{qp}")
                         for qp in range(QT)]
                x1sq7 = [x1sb_pool.tile([P, 2 * P], FP8, name=f"x1s7{qp}")
                         for qp in range(QT)]
                vp = [v_pool.tile([P, 2 * N], FP8, name=f"v{r}")
                      for r in range(QT)]
                expTp = [expT_pool.tile([P, 2 * C], FP8, name=f"eT{r}")
                         for r in range(QT)]
                attnA = []

                # ---- q GEMM (only needs consts; fills early PE idle) ----
                for nt in range(NT):
                    ps = ps_wide.tile([P, C], FP32)
                    for qp in range(QT):
                        lhsT = x2q[qp].rearrange(
                            "p (j x) -> p j x", j=2)[:, :, nt * P:(nt + 1) * P]
                        for off, w in _splits(C):
                            nc.tensor.matmul(
                                ps[:, off:off + w], lhsT,
                                wQp[qp][:, :, off:off + w],
                                start=(qp == 0), stop=(qp == QT - 1),
                                perf_mode=DRM)
                    nc.scalar.activation(
                        qTp[nt // 2][:, (nt % 2) * C:(nt % 2 + 1) * C],
                        ps[:], AF.Copy, scale=SCL_Q)

                def do_k(nt):
                    ps = ps_wide.tile([P, C], FP32)
                    for qp in range(QT):
                        if nt >= 6:
                            t = x1sq6 if nt == 6 else x1sq7
                            lhsT = pair(t[qp], P)
                        else:
                            lhsT = pair(x1sq[qp], N)[:, :, nt * P:(nt + 1) * P]
                        for off, w in _splits(C):
                            nc.tensor.matmul(
                                ps[:, off:off + w], lhsT,
                                wKp[qp][:, :, off:off + w],
                                start=(qp == 0), stop=(qp == QT - 1),
                                perf_mode=DRM)
                    nc.scalar.activation(
                        kTp[nt // 2][:, (nt % 2) * C:(nt % 2 + 1) * C],
                        ps[:], AF.Copy, scale=SCL_K)

                def do_v(noff, width, srcs=None, soff=0, swid=N):
                    for ot in range(CT):
                        for off, w in _splits(width):
                            ps = ps_half.tile([P, 512], FP32)
                            for qp in range(QT):
                                s = x1sq[qp] if srcs is None else srcs[qp]
                                nc.tensor.matmul(
                                    ps[:, :w],
                                    wVp[qp][:, :, ot * P:(ot + 1) * P],
                                    pair(s, swid)[
                                        :, :, soff + off:soff + off + w],
                                    start=(qp == 0), stop=(qp == QT - 1),
                                    perf_mode=DRM)
                            nc.scalar.activation(
                                vp[ot // 2][:, (ot % 2) * N + noff + off:
                                            (ot % 2) * N + noff + off + w],
                                ps[:, :w], AF.Copy, scale=SCL_V)

                for nt0, nnt in GROUPS:
                    rows = 16 * nnt
                    width = rows * SRC
                    for ct in range(CT):
                        st = stream_pool.tile([P, 16 * SRC * 4], FP8)
                        eng = nc.sync if ct % 2 == 0 else nc.scalar
                        eng.dma_start(
                            st[:, :width],
                            x1_d[ct * P:(ct + 1) * P,
                                 nt0 * 16:nt0 * 16 + rows, :],
                        )
                        # row-pool on PE via DoubleRow: each mm sums a
                        # row-pair; two mms per column-quad accumulate in PSUM
                        idp = id8p[:].rearrange("p (j m) -> p j m", j=2)
                        for k in range(nnt):
                            nt = nt0 + k
                            ps = ps_pool.tile([P, 512], FP32)
                            for q in range(4):
                                for s in range(2):
                                    off8 = (16 * k + 4 * q + 2 * s) * SRC
                                    rhs = st[:, off8:off8 + 2 * SRC].rearrange(
                                        "p (j w) -> p j w", j=2)
                                    nc.tensor.matmul(
                                        ps[:, q * P:(q + 1) * P], idp, rhs,
                                        start=(s == 0), stop=(s == 1),
                                        perf_mode=DRM,
                                    )
                            # pw-quad reduce psum -> fp8 pooled sums
                            if nt >= 6:
                                tgt = (x1sq6 if nt == 6 else x1sq7)[ct // 2]
                                o0 = (ct % 2) * P
                            else:
                                tgt = x1sq[ct // 2]
                                o0 = (ct % 2) * N + nt * P
                            srcv = ps[:].rearrange("p (x q) -> p x q", q=POOL)
                            with nc.allow_low_precision(
                                reason="pooled sums stored fp8 for the GEMMs"
                            ):
                                nc.vector.reduce_sum(
                                    tgt[:, o0:o0 + P], srcv, axis=AX.X)
                    for nt in range(nt0, nt0 + nnt):
                        do_k(nt)
                    if nt0 == 4:      # nt 0..3 pooled by now
                        do_v(0, 512)
                    if nt0 == 4:      # attn partial over nt0..5 (after k(5))
                        for c1t in range(CT):
                            ps = ps_wide.tile([P, C], FP32)
                            for i in range(3):
                                lhsT = pair(kTp[i], C)[
                                    :, :, c1t * P:(c1t + 1) * P]
                                for off, w in _splits(C):
                                    nc.tensor.matmul(
                                        ps[:, off:off + w], lhsT,
                                        pair(qTp[i], C)[:, :, off:off + w],
                                        start=(i == 0), stop=(i == 2),
                                        perf_mode=DRM)
                            at = attnA_pool.tile([P, C], BF16)
                            nc.scalar.activation(at[:], ps[:], AF.Copy)
                            attnA.append(at)
                    if nt0 == 6:      # nt 4,5 pooled long ago
                        do_v(512, 256)

                # ---- attn tail: psum = attnA (identity-fold) + nt6/nt7 ----
                for c1t in range(CT):
                    ps = ps_wide.tile([P, C], FP32)
                    for off, w in _splits(C):
                        nc.tensor.matmul(
                            ps[:, off:off + w], ident[:],
                            attnA[c1t][:, off:off + w],
                            start=True, stop=False, skip_group_check=True,
                        )
                    lhsT = pair(kTp[3], C)[:, :, c1t * P:(c1t + 1) * P]
                    for off, w in _splits(C):
                        nc.tensor.matmul(
                            ps[:, off:off + w], lhsT,
                            pair(qTp[3], C)[:, :, off:off + w],
                            start=False, stop=True, perf_mode=DRM,
                            skip_group_check=True)
                    nc.scalar.activation(
                        expTp[c1t // 2][:, (c1t % 2) * C:(c1t % 2 + 1) * C],
                        ps[:], AF.Exp, scale=SCL_E)
                do_v(768, 128, srcs=[t[:] for t in x1sq6], swid=P)
                do_v(896, 128, srcs=[t[:] for t in x1sq7], swid=P)

                # ---- out = (expT^T @ v) * (1/(16 s)) + x2 ----
                ones3 = onesp[:].rearrange("p (j o) -> p j o", j=2)[:, :, 0:1]
                for c2t in range(CT):
                    pss = ps_wide.tile([P, 1], FP32, name="ps")
                    for r in range(QT):
                        nc.tensor.matmul(
                            pss[:],
                            pair(expTp[r], C)[:, :, c2t * P:(c2t + 1) * P],
                            ones3,
                            start=(r == 0), stop=(r == QT - 1),
                            perf_mode=DRM)
                    rcp = rcp_pool.tile([P, 1], FP32)
                    nc.vector.reciprocal(rcp[:], pss[:])
                    for hf, (off, w) in enumerate(_splits(N)):
                        ps = ps_half.tile([P, 512], FP32)
                        for r in range(QT):
                            nc.tensor.matmul(
                                ps[:, :w],
                                pair(expTp[r], C)[
                                    :, :, c2t * P:(c2t + 1) * P],
                                pair(vp[r], N)[:, :, off:off + w],
                                start=(r == 0), stop=(r == QT - 1),
                                perf_mode=DRM)
                        o = out_pool.tile([P, 512], BF16)
                        if hf == 0:
                            o1 = out_pool.tile([P, 512], BF16, name="o1")
                            nc.scalar.activation(o1[:, :w], ps[:, :w],
                                                 AF.Copy, scale=rcp[:])
                            nc.vector.tensor_add(o[:, :w], o1[:, :w],
                                                 x2b[c2t][:, off:off + w])
                        else:
                            with nc.allow_low_precision(
                                reason="output shipped bf16"
                            ):
                                nc.vector.scalar_tensor_tensor(
                                    o[:, :w], ps[:, :w], rcp[:],
                                    x2b[c2t][:, off:off + w],
                                    op0=mybir.AluOpType.mult,
                                    op1=mybir.AluOpType.add)
                        nc.sync.dma_start(
                            out_d[c2t * P:(c2t + 1) * P, off:off + w], o[:, :w],
                        )

            if loop_reps is not None:
                with tc.For_i(0, loop_reps, 1,
                              hint_engines=(mybir.EngineType.PE,)):
                    for _ in range(reps):
                        body()
            else:
                for _ in range(reps):
                    body()

    nc.compile()
    return nc


_cache = {}


def _get_program(reps=1):
    if reps not in _cache:
        _cache[reps] = build_program(reps)
    return _cache[reps]


def _bf16(a):
    """fp32 -> bf16 with round-to-nearest-even (fast numpy path)."""
    u = np.ascontiguousarray(a, dtype=np.float32).view(np.uint32)
    r = ((u + 0x7FFF + ((u >> 16) & 1)) >> 16).astype(np.uint16)
    return r.view(ml_dtypes.bfloat16)


def _fold(a):
    """[CT*P, W] -> [P, CT*W] partition-fold (row ct*P+p -> partition p)."""
    w = a.shape[1]
    return a.reshape(CT, P, w).transpose(1, 0, 2).reshape(P, CT * w)


def _wpair8(wt):
    """[768(c_in), 768] fp32 -> [P, QT*2*768] fp8 pair-interleaved fold."""
    a = wt.reshape(QT, 2, P, C).transpose(2, 0, 1, 3).reshape(P, QT * 2 * C)
    return a.astype(ml_dtypes.float8_e4m3)


def kernel(x1, x2, Wq, Wk, Wv):
    B = x1.shape[0]
    assert B == NCORES
    nc = _get_program()

    wk = _wpair8(np.asarray(Wk, np.float32).T * (4.0 / 1.0))      # /16*64
    wv = _wpair8(np.asarray(Wv, np.float32).T * (4.0 / 1.0))
    wq = _wpair8(np.asarray(Wq, np.float32).T * (2.0 / 1.0))      # /32*64
    wts = np.ascontiguousarray(np.concatenate([wk, wv, wq], axis=1))

    x1b = np.asarray(x1, np.float32).astype(ml_dtypes.float8_e4m3)
    in_maps = []
    for b in range(B):
        x2r = np.asarray(x2[b], np.float32).reshape(C, N)
        x2f = np.ascontiguousarray(_fold(_bf16(x2r)))
        in_maps.append({
            "x1": np.ascontiguousarray(x1b[b]),
            "wts": wts,
            "x2b": x2f,
        })
    res = bass_utils.run_bass_kernel_spmd(nc, in_maps, core_ids=list(range(NCORES)))
    out = np.stack([np.asarray(res.results[b]["out"]).astype(np.float32)
                    .reshape(C, HW, HW) for b in range(B)])
    return out


# revision 32
# speedup vs baseline: 1.0256x; 1.0256x over previous
"""CrossViewFusion Trainium2 kernel (data-parallel over B=8 across 8 cores).

Per batch element:
  x1s = sum_pool4x4(x1)            [C,1024]  (pool /16 folded into Wk,Wv)
  qT  = x2f^T @ (Wq/32)^T          [1024,C]  (1/h attn scale folded into Wq)
  kT  = x1s^T @ (Wk/16)^T          [1024,C]
  v   = (Wv/16) @ x1s              [C,1024]
  aT  = exp(kT^T-contract-qT)      [C1,C2]   (softmax denom via ones-matmul,
  out = (aT^T @ v) * (1/s) + x2               applied post-GEMM as scale)

x1 is cast to bf16 on the host (halves the dominant HBM stream); weights are
pre-scaled, pre-transposed, and shipped in fp8e4 pair-interleaved layout.
All five GEMMs run as fp8 DoubleRow matmuls (two 128-deep contraction slices
per instruction), with power-of-2 rescales folded into the ACT psum->sbuf
converts so no fp8 operand lands in subnormal range.  The attention term is
~1% of the output magnitude, so fp8 staging keeps rel-err well under the
2e-2 gate.

Stream order is row-group-major (all 6 channel tiles per row group), so k is
fully contracted per n-tile as the stream lands.  Pooling runs as a
bf16 add-tree: row-pair stages hit the DVE 2x mode (GpSimd takes the two
earliest channel tiles' stages), and a final 1x pw-reduce writes fp8.  The
attn GEMM accumulates nt0..5 mid-stream; the partial is folded back into
PSUM via an identity-matmul so only nt6/nt7 + exp + out-GEMM sit in the
tail.  The last two row groups are half-size (16 rows) to shrink that tail.
"""

import sys
from contextlib import ExitStack

if "/opt/trn_rl_repo" not in sys.path:
    sys.path.insert(0, "/opt/trn_rl_repo")

import numpy as np
import ml_dtypes

import concourse.bass as bass
import concourse.tile as tile
from concourse import bacc, bass_utils, masks, mybir

FP32 = mybir.dt.float32
BF16 = mybir.dt.bfloat16
FP8 = mybir.dt.float8e4
DRM = mybir.MatmulPerfMode.DoubleRow
AX = mybir.AxisListType
AF = mybir.ActivationFunctionType

NCORES = 8

C = 768            # channels (C1 == C2)
P = 128            # partition size
CT = C // P        # channel tiles (6)
QT = CT // 2       # channel-tile pairs (3)
HW = 32            # pooled spatial side
N = HW * HW        # pooled spatial size (1024)
NT = N // P        # n-tiles (8); one n-tile == 16 source rows
SRC = 128          # source spatial side of x1
POOL = 4
# row groups of the x1 stream: 64 + 32 rows, then 2 x 16 rows (short tail)
GROUPS = [(0, 4), (4, 2), (6, 1), (7, 1)]  # (first nt, n nts)
# fp8 scale plan: shipped wk,wv = W.T/16*64, wq = W.T/32*64. ACT rescales:
SCL_K = 0.25       # kT_stored = 16*k   (sigma ~4 in fp8)
SCL_Q = 0.5        # qT_stored = 32*q   (sigma ~1)
SCL_V = 0.25       # v_stored  = 16*v   (sigma ~4)
SCL_E = 1.0 / 512  # attn psum = 512 * true logits -> exp(psum/512)
ONES_V = 16.0      # colsum ones = 16 -> pss = 16*s -> rcp = 1/(16 s)


def _splits(total, bank=512):
    off, out = 0, []
    while off < total:
        w = min(bank, total - off)
        out.append((off, w))
        off += w
    return out


def build_program(reps=1, loop_reps=None, timing_mode=False):
    """reps: python-unrolled repetitions. loop_reps: on-device For_i repetitions
    (for timing; same program size regardless of trip count). timing_mode makes
    the inputs Internal DRAM (uninitialized, nothing shipped per call)."""
    nc = bacc.Bacc("TRN2", target_bir_lowering=False, debug=False)

    kind = "Internal" if timing_mode else "ExternalInput"
    x1_d = nc.dram_tensor("x1", [C, SRC, SRC], FP8, kind=kind).ap()
    wts_d = nc.dram_tensor("wts", [P, 3 * CT * C], FP8, kind=kind).ap()
    x2_d = nc.dram_tensor("x2b", [P, CT * N], BF16, kind=kind).ap()
    out_d = nc.dram_tensor("out", [C, N], BF16, kind="ExternalOutput").ap()

    with tile.TileContext(nc) as tc:
        with ExitStack() as ctx:
            ent = ctx.enter_context
            const_pool = ent(tc.tile_pool(name="const", bufs=1))
            wts_pool = ent(tc.tile_pool(name="wts", bufs=1))
            x2_pool = ent(tc.tile_pool(name="x2p", bufs=1))
            x2q_pool = ent(tc.tile_pool(name="x2q", bufs=1))
            stream_pool = ent(tc.tile_pool(name="stream", bufs=8))
            tA_pool = ent(tc.tile_pool(name="tA", bufs=2))
            tB_pool = ent(tc.tile_pool(name="tB", bufs=2))
            x1sb_pool = ent(tc.tile_pool(name="x1sb", bufs=1))
            qT_pool = ent(tc.tile_pool(name="qT", bufs=1))
            kT_pool = ent(tc.tile_pool(name="kT", bufs=1))
            v_pool = ent(tc.tile_pool(name="vp", bufs=1))
            attnA_pool = ent(tc.tile_pool(name="attnA", bufs=CT))
            expT_pool = ent(tc.tile_pool(name="expT", bufs=1))
            rcp_pool = ent(tc.tile_pool(name="rcp", bufs=CT))
            out_pool = ent(tc.tile_pool(name="ost", bufs=4))
            ps_wide = ent(tc.tile_pool(name="ps_wide", bufs=2, space="PSUM"))
            ps_half = ent(tc.tile_pool(name="ps_half", bufs=2, space="PSUM"))
            ps_pool = ent(tc.tile_pool(name="ps_pool", bufs=2, space="PSUM"))

            ident = const_pool.tile([P, P], BF16)
            masks.make_identity(nc, ident[:])
            ident8 = const_pool.tile([P, P], FP8)
            nc.scalar.activation(ident8[:], ident[:], AF.Copy)
            id8p = const_pool.tile([P, 2 * P], FP8)
            nc.scalar.activation(id8p[:, 0:P], ident[:], AF.Copy)
            nc.scalar.activation(id8p[:, P:2 * P], ident[:], AF.Copy)
            onesp = const_pool.tile([P, 32], FP8)
            nc.gpsimd.memset(onesp[:], ONES_V)

            def pair(t, width):
                return t[:].rearrange("p (j x) -> p j x", j=2, x=width)

            def body():
                wts = wts_pool.tile([P, 3 * CT * C], FP8)
                nc.scalar.dma_start(wts[:], wts_d)
                x2b6 = x2_pool.tile([P, CT * N], BF16)
                nc.scalar.dma_start(x2b6[:], x2_d)
                x2b = [x2b6[:, ct * N:(ct + 1) * N] for ct in range(CT)]

                def wpair(w, qp):
                    off = (w * QT + qp) * 2 * C
                    return wts[:, off:off + 2 * C].rearrange(
                        "p (j c) -> p j c", j=2)
                wKp = [wpair(0, qp) for qp in range(QT)]
                wVp = [wpair(1, qp) for qp in range(QT)]
                wQp = [wpair(2, qp) for qp in range(QT)]

                # fp8 pair-interleaved copy of x2 for the q GEMM
                x2q6 = x2q_pool.tile([P, CT * N], FP8)
                for ct in range(CT):
                    nc.scalar.activation(
                        x2q6[:, (ct // 2) * 2 * N + (ct % 2) * N:
                             (ct // 2) * 2 * N + (ct % 2 + 1) * N],
                        x2b[ct], AF.Copy)
                x2q = [x2q6[:, qp * 2 * N:(qp + 1) * 2 * N]
                       for qp in range(QT)]

                qTp = [qT_pool.tile([P, 2 * C], FP8, name=f"qT{i}")
                       for i in range(NT // 2)]
                kTp = [kT_pool.tile([P, 2 * C], FP8, name=f"kT{i}")
                       for i in range(NT // 2)]
                x1sq = [x1sb_pool.tile([P, 2 * N], FP8, name=f"x1s{qp}")
                        for qp in range(QT)]
                x1sq6 = [x1sb_pool.tile([P, 2 * P], FP8, name=f"x1s6{qp}")
                         for qp in range(QT)]
                x1sq7 = [x1sb_pool.tile([P, 2 * P], FP8, name=f"x1s7{qp}")
                         for qp in range(QT)]
                vp = [v_pool.tile([P, 2 * N], FP8, name=f"v{r}")
                      for r in range(QT)]
                expTp = [expT_pool.tile([P, 2 * C], FP8, name=f"eT{r}")
                         for r in range(QT)]
                attnA = []

                # ---- q GEMM (only needs consts; fills early PE idle) ----
                for nt in range(NT):
                    ps = ps_wide.tile([P, C], FP32)
                    for qp in range(QT):
                        lhsT = x2q[qp].rearrange(
                            "p (j x) -> p j x", j=2)[:, :, nt * P:(nt + 1) * P]
                        for off, w in _splits(C):
                            nc.tensor.matmul(
                                ps[:, off:off + w], lhsT,
                                wQp[qp][:, :, off:off + w],
                                start=(qp == 0), stop=(qp == QT - 1),
                                perf_mode=DRM)
                    nc.scalar.activation(
                        qTp[nt // 2][:, (nt % 2) * C:(nt % 2 + 1) * C],
                        ps[:], AF.Copy, scale=SCL_Q)

                def do_k(nt):
                    ps = ps_wide.tile([P, C], FP32)
                    for qp in range(QT):
                        if nt >= 6:
                            t = x1sq6 if nt == 6 else x1sq7
                            lhsT = pair(t[qp], P)
                        else:
                            lhsT = pair(x1sq[qp], N)[:, :, nt * P:(nt + 1) * P]
                        for off, w in _splits(C):
                            nc.tensor.matmul(
                                ps[:, off:off + w], lhsT,
                                wKp[qp][:, :, off:off + w],
                                start=(qp == 0), stop=(qp == QT - 1),
                                perf_mode=DRM)
                    nc.scalar.activation(
                        kTp[nt // 2][:, (nt % 2) * C:(nt % 2 + 1) * C],
                        ps[:], AF.Copy, scale=SCL_K)

                def do_v(noff, width, srcs=None, soff=0, swid=N):
                    for ot in range(CT):
                        for off, w in _splits(width):
                            ps = ps_half.tile([P, 512], FP32)
                            for qp in range(QT):
                                s = x1sq[qp] if srcs is None else srcs[qp]
                                nc.tensor.matmul(
                                    ps[:, :w],
                                    wVp[qp][:, :, ot * P:(ot + 1) * P],
                                    pair(s, swid)[
                                        :, :, soff + off:soff + off + w],
                                    start=(qp == 0), stop=(qp == QT - 1),
                                    perf_mode=DRM)
                            nc.scalar.activation(
                                vp[ot // 2][:, (ot % 2) * N + noff + off:
                                            (ot % 2) * N + noff + off + w],
                                ps[:, :w], AF.Copy, scale=SCL_V)

                for nt0, nnt in GROUPS:
                    rows = 16 * nnt
                    width = rows * SRC
                    for ct in range(CT):
                        st = stream_pool.tile([P, 16 * SRC * 4], FP8)
                        nc.sync.dma_start(
                            st[:, :width],
                            x1_d[ct * P:(ct + 1) * P,
                                 nt0 * 16:nt0 * 16 + rows, :],
                        )
                        # row-pool on PE via DoubleRow: each mm sums a
                        # row-pair; two mms per column-quad accumulate in PSUM
                        idp = id8p[:].rearrange("p (j m) -> p j m", j=2)
                        for k in range(nnt):
                            nt = nt0 + k
                            ps = ps_pool.tile([P, 512], FP32)
                            for q in range(4):
                                for s in range(2):
                                    off8 = (16 * k + 4 * q + 2 * s) * SRC
                                    rhs = st[:, off8:off8 + 2 * SRC].rearrange(
                                        "p (j w) -> p j w", j=2)
                                    nc.tensor.matmul(
                                        ps[:, q * P:(q + 1) * P], idp, rhs,
                                        start=(s == 0), stop=(s == 1),
                                        perf_mode=DRM,
                                    )
                            # pw-quad reduce psum -> fp8 pooled sums
                            if nt >= 6:
                                tgt = (x1sq6 if nt == 6 else x1sq7)[ct // 2]
                                o0 = (ct % 2) * P
                            else:
                                tgt = x1sq[ct // 2]
                                o0 = (ct % 2) * N + nt * P
                            srcv = ps[:].rearrange("p (x q) -> p x q", q=POOL)
                            with nc.allow_low_precision(
                                reason="pooled sums stored fp8 for the GEMMs"
                            ):
                                nc.vector.reduce_sum(
                                    tgt[:, o0:o0 + P], srcv, axis=AX.X)
                    for nt in range(nt0, nt0 + nnt):
                        do_k(nt)
                    if nt0 == 4:      # nt 0..3 pooled by now
                        do_v(0, 512)
                    if nt0 == 4:      # attn partial over nt0..5 (after k(5))
                        for c1t in range(CT):
                            ps = ps_wide.tile([P, C], FP32)
                            for i in range(3):
                                lhsT = pair(kTp[i], C)[
                                    :, :, c1t * P:(c1t + 1) * P]
                                for off, w in _splits(C):
                                    nc.tensor.matmul(
                                        ps[:, off:off + w], lhsT,
                                        pair(qTp[i], C)[:, :, off:off + w],
                                        start=(i == 0), stop=(i == 2),
                                        perf_mode=DRM)
                            at = attnA_pool.tile([P, C], BF16)
                            nc.scalar.activation(at[:], ps[:], AF.Copy)
                            attnA.append(at)
                    if nt0 == 6:      # nt 4,5 pooled long ago
                        do_v(512, 256)

                # ---- attn tail: psum = attnA (identity-fold) + nt6/nt7 ----
                for c1t in range(CT):
                    ps = ps_wide.tile([P, C], FP32)
                    for off, w in _splits(C):
                        nc.tensor.matmul(
                            ps[:, off:off + w], ident[:],
                            attnA[c1t][:, off:off + w],
                            start=True, stop=False, skip_group_check=True,
                        )
                    lhsT = pair(kTp[3], C)[:, :, c1t * P:(c1t + 1) * P]
                    for off, w in _splits(C):
                        nc.tensor.matmul(
                            ps[:, off:off + w], lhsT,
                            pair(qTp[3], C)[:, :, off:off + w],
                            start=False, stop=True, perf_mode=DRM,
                            skip_group_check=True)
                    nc.scalar.activation(
                        expTp[c1t // 2][:, (c1t % 2) * C:(c1t % 2 + 1) * C],
                        ps[:], AF.Exp, scale=SCL_E)
                do_v(768, 128, srcs=[t[:] for t in x1sq6], swid=P)
                do_v(896, 128, srcs=[t[:] for t in x1sq7], swid=P)

                # ---- out = (expT^T @ v) * (1/(16 s)) + x2 ----
                ones3 = onesp[:].rearrange("p (j o) -> p j o", j=2)[:, :, 0:1]
                for c2t in range(CT):
                    pss = ps_wide.tile([P, 1], FP32, name="ps")
                    for r in range(QT):
                        nc.tensor.matmul(
                            pss[:],
                            pair(expTp[r], C)[:, :, c2t * P:(c2t + 1) * P],
                            ones3,
                            start=(r == 0), stop=(r == QT - 1),
                            perf_mode=DRM)
                    rcp = rcp_pool.tile([P, 1], FP32)
                    nc.vector.reciprocal(rcp[:], pss[:])
                    for hf, (off, w) in enumerate(_splits(N)):
                        ps = ps_half.tile([P, 512], FP32)
                        for r in range(QT):
                            nc.tensor.matmul(
                                ps[:, :w],
                                pair(expTp[r], C)[
                                    :, :, c2t * P:(c2t + 1) * P],
                                pair(vp[r], N)[:, :, off:off + w],
                                start=(r == 0), stop=(r == QT - 1),
                                perf_mode=DRM)
                        o = out_pool.tile([P, 512], BF16)
                        if hf == 0:
                            o1 = out_pool.tile([P, 512], BF16, name="o1")
                            nc.scalar.activation(o1[:, :w], ps[:, :w],
                                                 AF.Copy, scale=rcp[:])
                            nc.vector.tensor_add(o[:, :w], o1[:, :w],
                                                 x2b[c2t][:, off:off + w])
                        else:
                            with nc.allow_low_precision(
                                reason="output shipped bf16"
                            ):
                                nc.vector.scalar_tensor_tensor(
                                    o[:, :w], ps[:, :w], rcp[:],
                                    x2b[c2t][:, off:off + w],
                                    op0=mybir.AluOpType.mult,
                                    op1=mybir.AluOpType.add)
                        nc.sync.dma_start(
                            out_d[c2t * P:(c2t + 1) * P, off:off + w], o[:, :w],
                        )

            if loop_reps is not None:
                with tc.For_i(0, loop_reps, 1,
                              hint_engines=(mybir.EngineType.PE,)):
                    for _ in range(reps):
                        body()
            else:
                for _ in range(reps):
                    body()

    nc.compile()
    return nc


_cache = {}


def _get_program(reps=1):
    if reps not in _cache:
        _cache[reps] = build_program(reps)
    return _cache[reps]


def _bf16(a):
    """fp32 -> bf16 with round-to-nearest-even (fast numpy path)."""
    u = np.ascontiguousarray(a, dtype=np.float32).view(np.uint32)
    r = ((u + 0x7FFF + ((u >> 16) & 1)) >> 16).astype(np.uint16)
    return r.view(ml_dtypes.bfloat16)


def _fold(a):
    """[CT*P, W] -> [P, CT*W] partition-fold (row ct*P+p -> partition p)."""
    w = a.shape[1]
    return a.reshape(CT, P, w).transpose(1, 0, 2).reshape(P, CT * w)


def _wpair8(wt):
    """[768(c_in), 768] fp32 -> [P, QT*2*768] fp8 pair-interleaved fold."""
    a = wt.reshape(QT, 2, P, C).transpose(2, 0, 1, 3).reshape(P, QT * 2 * C)
    return a.astype(ml_dtypes.float8_e4m3)


def kernel(x1, x2, Wq, Wk, Wv):
    B = x1.shape[0]
    assert B == NCORES
    nc = _get_program()

    wk = _wpair8(np.asarray(Wk, np.float32).T * (4.0 / 1.0))      # /16*64
    wv = _wpair8(np.asarray(Wv, np.float32).T * (4.0 / 1.0))
    wq = _wpair8(np.asarray(Wq, np.float32).T * (2.0 / 1.0))      # /32*64
    wts = np.ascontiguousarray(np.concatenate([wk, wv, wq], axis=1))

    x1b = np.asarray(x1, np.float32).astype(ml_dtypes.float8_e4m3)
    in_maps = []
    for b in range(B):
        x2r = np.asarray(x2[b], np.float32).reshape(C, N)
        x2f = np.ascontiguousarray(_fold(_bf16(x2r)))
        in_maps.append({
            "x1": np.ascontiguousarray(x1b[b]),
            "wts": wts,
            "x2b": x2f,
        })
    res = bass_utils.run_bass_kernel_spmd(nc, in_maps, core_ids=list(range(NCORES)))
    out = np.stack([np.asarray(res.results[b]["out"]).astype(np.float32)
                    .reshape(C, HW, HW) for b in range(B)])
    return out
